# revision 27
# baseline (speedup 1.0000x reference)
"""Trainium2 Bass kernel for nn_MultiHeadAttention_85761906966848 (sparse_attention).

Diagonal-only attention: out[b,s,:] = (softmax(masked scores)[s,s] * v) @ W0 + b0.
Data-parallel over batch (core b computes batch b end-to-end), as v1.

v3: same math/idioms as v1, restructured so the ScalarE exp stream (the
~130us critical resource) starts at ~15us instead of ~85us: Q and K project
PER DIM-CHUNK with that chunk's scores + tril + exp+accum emitted immediately
after, so exp overlaps the remaining projections.  Wq and Wk are both resident
(Wv/W0 rotate in later); V-proj runs during the scalar lag and writes into
Wq's dead buffers; the diag accumulates in SBUF to keep matmul accumulation
groups contiguous.
"""

import numpy as np
import concourse.bass as bass
import concourse.bacc as bacc
import concourse.mybir as mybir
from concourse import tile

F32 = mybir.dt.float32
F32R = mybir.dt.float32r
AF = mybir.ActivationFunctionType

B, S, D, H = 8, 1024, 1024, 16
NEG = -1.0e30

_CACHE = {}


def blocks(total, width=512):
    out = []
    off = 0
    while off < total:
        w = min(width, total - off)
        out.append((off, w))
        off += w
    return out


def _build(S=1024, D=1024, H=16):
    dk = D // H
    C = D // 128          # number of 128-row d-chunks
    T = S // 128          # number of 128-row s-tiles
    HPC = 128 // dk       # heads per chunk
    assert dk * H == D and C * 128 == D and T * 128 == S and HPC * dk == 128

    MMDT = F32R

    nc = bacc.Bacc("TRN2", target_bir_lowering=False, debug=False, num_devices=8)

    xt_d = nc.dram_tensor("xt", [D, S], MMDT, kind="ExternalInput")
    w_d = {}
    for wn in ("wq", "wk", "wv", "w0"):
        w_d[wn] = nc.dram_tensor(wn, [D, D], MMDT, kind="ExternalInput")
    bqt_d = nc.dram_tensor("bqt", [128, C], F32, kind="ExternalInput")
    bkt_d = nc.dram_tensor("bkt", [128, C], F32, kind="ExternalInput")
    bvt_d = nc.dram_tensor("bvt", [128, C], F32, kind="ExternalInput")
    b0b_d = nc.dram_tensor("b0b", [128, D], F32, kind="ExternalInput")
    ed_d = nc.dram_tensor("ed", [C, 128, H], MMDT, kind="ExternalInput")
    ebc_d = nc.dram_tensor("ebc", [C, H, 128], MMDT, kind="ExternalInput")
    mask_d = nc.dram_tensor("maskh", [H, S], F32, kind="ExternalInput")
    tril_d = nc.dram_tensor("trilneg", [128, 128], F32, kind="ExternalInput")
    iden_d = nc.dram_tensor("iden", [128, 128], F32, kind="ExternalInput")
    out_d = nc.dram_tensor("out", [S, D], F32, kind="ExternalOutput")

    with tile.TileContext(nc) as tc:
        with (
            tc.tile_pool(name="cp", bufs=1) as cp,
            tc.tile_pool(name="xtp", bufs=1) as xtp,
            tc.tile_pool(name="wpa", bufs=C) as wpa,
            tc.tile_pool(name="wpb", bufs=C) as wpb,
            tc.tile_pool(name="qkp", bufs=1) as qkp,
            tc.tile_pool(name="prodp", bufs=1) as prodp,
            tc.tile_pool(name="qzp", bufs=2) as qzp,
            tc.tile_pool(name="outp", bufs=2) as outp,
            tc.tile_pool(name="pp", bufs=2, space=bass.MemorySpace.PSUM) as pp,
            tc.tile_pool(name="pbig", bufs=3, space=bass.MemorySpace.PSUM) as pbig,
        ):
            # ---------------- constants ----------------
            iden = cp.tile([128, 128], F32, tag="iden")
            nc.sync.dma_start(iden[:], iden_d[:])
            tril = cp.tile([128, 128], F32, tag="tril")
            nc.sync.dma_start(tril[:], tril_d[:])
            b0b = cp.tile([128, D], F32, tag="b0b")
            nc.sync.dma_start(b0b[:], b0b_d[:])
            bqt = cp.tile([128, C], F32, tag="bqt")
            nc.sync.dma_start(bqt[:], bqt_d[:])
            bkt = cp.tile([128, C], F32, tag="bkt")
            nc.sync.dma_start(bkt[:], bkt_d[:])
            bvt = cp.tile([128, C], F32, tag="bvt")
            nc.sync.dma_start(bvt[:], bvt_d[:])
            maskh = cp.tile([H, S], F32, tag="maskh")
            nc.sync.dma_start(maskh[:], mask_d[:])
            ed = []
            ebc = []
            for c in range(C):
                e1 = cp.tile([128, H], MMDT, name=f"ed{c}", tag=f"ed{c}")
                nc.sync.dma_start(e1[:], ed_d[c, :, :])
                ed.append(e1)
                e2 = cp.tile([H, 128], MMDT, name=f"ebc{c}", tag=f"ebc{c}")
                nc.sync.dma_start(e2[:], ebc_d[c, :, :])
                ebc.append(e2)

            # persistent small result tiles
            diag_exp = cp.tile([H, S], F32, tag="diag_exp")
            denomT = cp.tile([H, S], F32, tag="denomT")
            arec = cp.tile([H, S], F32, tag="arec")
            a_t = cp.tile([H, S], MMDT, tag="a_t")
            dgacc = denomT  # diag accumulator is dead before denomT is written
            dn = [cp.tile([128, H], F32, name=f"dn{i}", tag=f"dn{i}") for i in range(T)]

            # ---------------- X^T load (host pre-transposed) ----------------
            xt = [xtp.tile([128, S], MMDT, name=f"xt{c}", tag=f"xt{c}") for c in range(C)]
            for (off, wd) in blocks(S):
                for c in range(C):
                    nc.sync.dma_start(xt[c][:, off:off + wd],
                                      xt_d[c * 128:(c + 1) * 128, off:off + wd])

            def load_w(pool, name):
                wts = [pool.tile([128, D], MMDT, name=f"{name}{c}", tag="w")
                       for c in range(C)]
                for (off, wd) in blocks(D):
                    for c in range(C):
                        nc.scalar.dma_start(
                            wts[c][:, off:off + wd],
                            w_d[name][c * 128:(c + 1) * 128, off:off + wd])
                return wts

            wq = load_w(wpa, "wq")
            wk = load_w(wpb, "wk")

            def proj_chunk(wts, dd, bias_tile, dst):
                for (off, wd) in blocks(S):
                    ps = pp.tile([128, 512], F32, tag="mm")
                    for kk in range(C):
                        nc.tensor.matmul(
                            ps[:, 0:wd],
                            wts[kk][:, dd * 128:(dd + 1) * 128],
                            xt[kk][:, off:off + wd],
                            start=(kk == 0),
                            stop=(kk == C - 1),
                        )
                    nc.vector.tensor_scalar_add(
                        dst[:, off:off + wd], ps[:, 0:wd],
                        bias_tile[:, dd:dd + 1])

            qt = [qkp.tile([128, S], MMDT, name=f"q{c}", tag=f"q{c}")
                  for c in range(C)]
            kt = [qkp.tile([128, S], MMDT, name=f"k{c}", tag=f"k{c}")
                  for c in range(C)]

            # ------- fused per-chunk: Q, K proj -> diag partials -> scores ---
            for c in range(C):
                proj_chunk(wq, c, bqt, qt[c])
                proj_chunk(wk, c, bkt, kt[c])

                # diag partial: dgacc += ed[c]^T @ (qt[c] * kt[c])
                for (off, wd) in blocks(S):
                    pr = prodp.tile([128, 512], MMDT, tag="prod")
                    nc.vector.tensor_mul(
                        pr[:, 0:wd], qt[c][:, off:off + wd], kt[c][:, off:off + wd])
                    dgc = pp.tile([128, 512], F32, tag="mm")
                    nc.tensor.matmul(
                        dgc[0:H, 0:wd],
                        ed[c][:],
                        pr[:, 0:wd],
                        start=True,
                        stop=True,
                    )
                    if c == 0:
                        nc.vector.tensor_copy(dgacc[:, off:off + wd],
                                              dgc[0:H, 0:wd])
                    else:
                        nc.vector.tensor_add(dgacc[:, off:off + wd],
                                             dgacc[:, off:off + wd],
                                             dgc[0:H, 0:wd])

                # scores for this chunk's two heads, zero-padded to K=128
                qzs = []
                for p in range(HPC):
                    qz = qzp.tile([128, S], MMDT, name=f"qz{p}", tag="qz")
                    zo = (1 - p) * dk
                    nc.vector.tensor_scalar_mul(qz[zo:zo + dk, :], qt[c][zo:zo + dk, :], 0.0)
                    nc.vector.tensor_copy(
                        qz[p * dk:(p + 1) * dk, :], qt[c][p * dk:(p + 1) * dk, :])
                    qzs.append(qz)
                for i in range(T):
                    N = (i + 1) * 128
                    for p in range(HPC):
                        h = c * HPC + p
                        sc = pbig.tile([128, 1024], F32, name=f"sc{p}", tag="big")
                        for (off, wd) in blocks(N):
                            nc.tensor.matmul(
                                sc[:, off:off + wd],
                                qzs[p][:, i * 128:(i + 1) * 128],
                                kt[c][:, off:off + wd],
                                start=True,
                                stop=True,
                            )
                        nc.vector.tensor_add(
                            sc[:, i * 128:N], sc[:, i * 128:N], tril[:])
                        nc.scalar.activation(
                            sc[:, 0:N], sc[:, 0:N], AF.Exp,
                            accum_out=dn[i][:, h:h + 1])

            # ---------------- V projection (during the exp lag) -------------
            wv = load_w(wpb, "wv")
            vt = [wpa.tile([128, S], MMDT, name=f"v{c}", tag="w")
                  for c in range(C)]
            for c in range(C):
                proj_chunk(wv, c, bvt, vt[c])

            w0 = load_w(wpb, "w0")

            # ---------------- diag + denominators -> a ----------------
            nc.scalar.activation(diag_exp[:], dgacc[:], AF.Exp)
            nc.vector.tensor_mul(diag_exp[:], diag_exp[:], maskh[:])
            for i in range(T):
                tp = pp.tile([128, 512], F32, tag="mm")
                nc.tensor.transpose(tp[0:H, 0:128], dn[i][:], iden[:])
                nc.vector.tensor_copy(denomT[:, i * 128:(i + 1) * 128], tp[0:H, 0:128])
            nc.vector.reciprocal(arec[:], denomT[:])
            nc.vector.tensor_mul(a_t[:], diag_exp[:], arec[:])

            # ---------------- diagonal weighting of V ----------------
            for c in range(C):
                for (off, wd) in blocks(S):
                    abp = pp.tile([128, 512], F32, tag="mm")
                    nc.tensor.matmul(
                        abp[:, 0:wd],
                        ebc[c][:],
                        a_t[:, off:off + wd],
                        start=True,
                        stop=True,
                    )
                    nc.vector.tensor_mul(vt[c][:, off:off + wd],
                                         vt[c][:, off:off + wd], abp[:, 0:wd])

            # ---------------- output projection ----------------
            for m in range(T):
                for (off, wd) in blocks(D):
                    ps = pp.tile([128, 512], F32, tag="mm")
                    for c in range(C):
                        nc.tensor.matmul(
                            ps[:, 0:wd],
                            vt[c][:, m * 128:(m + 1) * 128],
                            w0[c][:, off:off + wd],
                            start=(c == 0),
                            stop=(c == C - 1),
                        )
                    ot = outp.tile([128, 512], F32, tag="o")
                    nc.vector.tensor_add(ot[:, 0:wd], ps[:, 0:wd], b0b[:, off:off + wd])
                    nc.sync.dma_start(
                        out_d[m * 128:(m + 1) * 128, off:off + wd], ot[:, 0:wd])

    nc.compile()
    return nc


def _get_nc():
    if "nc" not in _CACHE:
        _CACHE["nc"] = _build(S, D, H)
    return _CACHE["nc"]


def _host_aux(length):
    dk = D // H
    C = D // 128
    aux = {}
    aux["iden"] = np.eye(128, dtype=np.float32)
    tril = np.zeros((128, 128), np.float32)
    tril[np.triu_indices(128, 1)] = NEG
    aux["trilneg"] = tril
    ed = np.zeros((C, 128, H), np.float32)
    ebc = np.zeros((C, H, 128), np.float32)
    for c in range(C):
        for dl in range(128):
            h = (c * 128 + dl) // dk
            ed[c, dl, h] = 1.0
            ebc[c, h, dl] = 1.0
    aux["ed"] = ed
    aux["ebc"] = ebc
    mask = (np.arange(S) < int(length)).astype(np.float32)
    aux["maskh"] = np.tile(mask[None, :], (H, 1))
    return aux


def _in_map(x, wq, bq, wk, bk, wv, bv, w0, b0, length):
    C = D // 128
    inp = {"xt": np.ascontiguousarray(np.asarray(x, np.float32).T)}
    inp["wq"] = np.ascontiguousarray(wq, np.float32)
    inp["wk"] = np.ascontiguousarray(wk, np.float32)
    inp["wv"] = np.ascontiguousarray(wv, np.float32)
    inp["w0"] = np.ascontiguousarray(w0, np.float32)
    inp["bqt"] = np.ascontiguousarray(np.asarray(bq, np.float32).reshape(C, 128).T)
    inp["bkt"] = np.ascontiguousarray(np.asarray(bk, np.float32).reshape(C, 128).T)
    inp["bvt"] = np.ascontiguousarray(np.asarray(bv, np.float32).reshape(C, 128).T)
    inp["b0b"] = np.ascontiguousarray(
        np.tile(np.asarray(b0, np.float32)[None, :], (128, 1)))
    inp.update(_host_aux(length))
    return inp


def _run(inputs, trace=False):
    from concourse.bass_utils import run_bass_kernel_spmd

    batch = np.asarray(inputs["batch"], np.float32)
    lengths = np.asarray(inputs["lengths"])
    nb = batch.shape[0]
    assert batch.shape[1:] == (S, D), batch.shape
    nc = _get_nc()
    in_maps = [
        _in_map(batch[b], inputs["wq"], inputs["bq"], inputs["wk"], inputs["bk"],
                inputs["wv"], inputs["bv"], inputs["w0"], inputs["b0"],
                int(lengths[b]))
        for b in range(nb)
    ]
    res = run_bass_kernel_spmd(nc, in_maps, core_ids=list(range(nb)), trace=trace)
    out = np.stack([r["out"] for r in res.results]).astype(np.float32)
    return out, res


def kernel(**inputs) -> np.ndarray:
    out, _ = _run(inputs, trace=False)
    return out


# revision 30
# speedup vs baseline: 1.0502x; 1.0502x over previous
"""Trainium2 Bass kernel for nn_MultiHeadAttention_85761906966848 (sparse_attention).

Diagonal-only attention: out[b,s,:] = (softmax(masked scores)[s,s] * v) @ W0 + b0.
Data-parallel over batch (core b computes batch b end-to-end), as v1.

v3: same math/idioms as v1, restructured so the ScalarE exp stream (the
~130us critical resource) starts at ~15us instead of ~85us: Q and K project
PER DIM-CHUNK with that chunk's scores + tril + exp+accum emitted immediately
after, so exp overlaps the remaining projections.  Wq and Wk are both resident
(Wv/W0 rotate in later); V-proj runs during the scalar lag and writes into
Wq's dead buffers; the diag accumulates in SBUF to keep matmul accumulation
groups contiguous.
"""

import numpy as np
import concourse.bass as bass
import concourse.bacc as bacc
import concourse.mybir as mybir
from concourse import tile

F32 = mybir.dt.float32
F32R = mybir.dt.float32r
AF = mybir.ActivationFunctionType

B, S, D, H = 8, 1024, 1024, 16
NEG = -1.0e30

_CACHE = {}


def blocks(total, width=512):
    out = []
    off = 0
    while off < total:
        w = min(width, total - off)
        out.append((off, w))
        off += w
    return out


def _build(S=1024, D=1024, H=16):
    dk = D // H
    C = D // 128          # number of 128-row d-chunks
    T = S // 128          # number of 128-row s-tiles
    HPC = 128 // dk       # heads per chunk
    assert dk * H == D and C * 128 == D and T * 128 == S and HPC * dk == 128

    MMDT = F32R

    nc = bacc.Bacc("TRN2", target_bir_lowering=False, debug=False, num_devices=8)

    xt_d = nc.dram_tensor("xt", [D, S], MMDT, kind="ExternalInput")
    w_d = {}
    for wn in ("wq", "wk", "wv", "w0"):
        w_d[wn] = nc.dram_tensor(wn, [D, D], MMDT, kind="ExternalInput")
    bqt_d = nc.dram_tensor("bqt", [128, C], F32, kind="ExternalInput")
    bkt_d = nc.dram_tensor("bkt", [128, C], F32, kind="ExternalInput")
    bvt_d = nc.dram_tensor("bvt", [128, C], F32, kind="ExternalInput")
    b0b_d = nc.dram_tensor("b0b", [128, D], F32, kind="ExternalInput")
    ed_d = nc.dram_tensor("ed", [C, 128, H], MMDT, kind="ExternalInput")
    ebc_d = nc.dram_tensor("ebc", [C, H, 128], MMDT, kind="ExternalInput")
    mask_d = nc.dram_tensor("maskh", [H, S], F32, kind="ExternalInput")
    tril_d = nc.dram_tensor("trilneg", [128, 128], F32, kind="ExternalInput")
    iden_d = nc.dram_tensor("iden", [128, 128], F32, kind="ExternalInput")
    out_d = nc.dram_tensor("out", [S, D], F32, kind="ExternalOutput")

    with tile.TileContext(nc) as tc:
        with (
            tc.tile_pool(name="cp", bufs=1) as cp,
            tc.tile_pool(name="xtp", bufs=1) as xtp,
            tc.tile_pool(name="wpa", bufs=C) as wpa,
            tc.tile_pool(name="wpb", bufs=C) as wpb,
            tc.tile_pool(name="qkp", bufs=1) as qkp,
            tc.tile_pool(name="prodp", bufs=1) as prodp,
            tc.tile_pool(name="qzp", bufs=1) as qzp,
            tc.tile_pool(name="outp", bufs=2) as outp,
            tc.tile_pool(name="pp", bufs=2, space=bass.MemorySpace.PSUM) as pp,
            tc.tile_pool(name="pbig", bufs=3, space=bass.MemorySpace.PSUM) as pbig,
        ):
            # ---------------- constants ----------------
            iden = cp.tile([128, 128], F32, tag="iden")
            nc.sync.dma_start(iden[:], iden_d[:])
            tril = cp.tile([128, 128], F32, tag="tril")
            nc.sync.dma_start(tril[:], tril_d[:])
            b0b = cp.tile([128, D], F32, tag="b0b")
            nc.sync.dma_start(b0b[:], b0b_d[:])
            bqt = cp.tile([128, C], F32, tag="bqt")
            nc.sync.dma_start(bqt[:], bqt_d[:])
            bkt = cp.tile([128, C], F32, tag="bkt")
            nc.sync.dma_start(bkt[:], bkt_d[:])
            bvt = cp.tile([128, C], F32, tag="bvt")
            nc.sync.dma_start(bvt[:], bvt_d[:])
            maskh = cp.tile([H, S], F32, tag="maskh")
            nc.sync.dma_start(maskh[:], mask_d[:])
            ed = []
            ebc = []
            for c in range(C):
                e1 = cp.tile([128, H], MMDT, name=f"ed{c}", tag=f"ed{c}")
                nc.sync.dma_start(e1[:], ed_d[c, :, :])
                ed.append(e1)
                e2 = cp.tile([H, 128], MMDT, name=f"ebc{c}", tag=f"ebc{c}")
                nc.sync.dma_start(e2[:], ebc_d[c, :, :])
                ebc.append(e2)

            # persistent small result tiles
            diag_exp = cp.tile([H, S], F32, tag="diag_exp")
            denomT = cp.tile([H, S], F32, tag="denomT")
            arec = cp.tile([H, S], F32, tag="arec")
            a_t = cp.tile([H, S], MMDT, tag="a_t")
            dgacc = denomT  # diag accumulator is dead before denomT is written
            dn = [cp.tile([128, H], F32, name=f"dn{i}", tag=f"dn{i}") for i in range(T)]

            # ---------------- X^T load (host pre-transposed) ----------------
            xt = [xtp.tile([128, S], MMDT, name=f"xt{c}", tag=f"xt{c}") for c in range(C)]
            for (off, wd) in blocks(S):
                for c in range(C):
                    nc.sync.dma_start(xt[c][:, off:off + wd],
                                      xt_d[c * 128:(c + 1) * 128, off:off + wd])

            def load_w(pool, name):
                wts = [pool.tile([128, D], MMDT, name=f"{name}{c}", tag="w")
                       for c in range(C)]
                for (off, wd) in blocks(D):
                    for c in range(C):
                        nc.scalar.dma_start(
                            wts[c][:, off:off + wd],
                            w_d[name][c * 128:(c + 1) * 128, off:off + wd])
                return wts

            wq = load_w(wpa, "wq")
            wk = load_w(wpb, "wk")

            def proj_chunk(wts, dd, bias_tile, dst):
                for (off, wd) in blocks(S):
                    ps = pp.tile([128, 512], F32, tag="mm")
                    for kk in range(C):
                        nc.tensor.matmul(
                            ps[:, 0:wd],
                            wts[kk][:, dd * 128:(dd + 1) * 128],
                            xt[kk][:, off:off + wd],
                            start=(kk == 0),
                            stop=(kk == C - 1),
                        )
                    nc.vector.tensor_scalar_add(
                        dst[:, off:off + wd], ps[:, 0:wd],
                        bias_tile[:, dd:dd + 1])

            qt = [qkp.tile([128, S], MMDT, name=f"q{c}", tag=f"q{c}")
                  for c in range(C)]
            kt = [qkp.tile([128, S], MMDT, name=f"k{c}", tag=f"k{c}")
                  for c in range(C)]

            # persistent zero-padded Q staging: the dead half of each buffer
            # is zeroed once and never overwritten afterwards
            qzper = []
            for p in range(HPC):
                qz = qzp.tile([128, S], MMDT, name=f"qzp{p}", tag=f"qz{p}")
                zo = (1 - p) * dk
                nc.vector.tensor_scalar_mul(qz[zo:zo + dk, :],
                                            xt[0][zo:zo + dk, :], 0.0)
                qzper.append(qz)

            # ------- fused per-chunk: Q, K proj -> diag partials -> scores ---
            for c in range(C):
                proj_chunk(wq, c, bqt, qt[c])
                proj_chunk(wk, c, bkt, kt[c])

                # diag partial: dgacc += ed[c]^T @ (qt[c] * kt[c])
                for (off, wd) in blocks(S):
                    pr = prodp.tile([128, 512], MMDT, tag="prod")
                    nc.vector.tensor_mul(
                        pr[:, 0:wd], qt[c][:, off:off + wd], kt[c][:, off:off + wd])
                    dgc = pp.tile([128, 512], F32, tag="mm")
                    nc.tensor.matmul(
                        dgc[0:H, 0:wd],
                        ed[c][:],
                        pr[:, 0:wd],
                        start=True,
                        stop=True,
                    )
                    if c == 0:
                        nc.vector.tensor_copy(dgacc[:, off:off + wd],
                                              dgc[0:H, 0:wd])
                    else:
                        nc.vector.tensor_add(dgacc[:, off:off + wd],
                                             dgacc[:, off:off + wd],
                                             dgc[0:H, 0:wd])

                # scores for this chunk's two heads, zero-padded to K=128
                qzs = qzper
                for p in range(HPC):
                    nc.vector.tensor_copy(
                        qzs[p][p * dk:(p + 1) * dk, :],
                        qt[c][p * dk:(p + 1) * dk, :])
                for i in range(T):
                    N = (i + 1) * 128
                    for p in range(HPC):
                        h = c * HPC + p
                        sc = pbig.tile([128, 1024], F32, name=f"sc{p}", tag="big")
                        for (off, wd) in blocks(N):
                            nc.tensor.matmul(
                                sc[:, off:off + wd],
                                qzs[p][:, i * 128:(i + 1) * 128],
                                kt[c][:, off:off + wd],
                                start=True,
                                stop=True,
                            )
                        nc.vector.tensor_add(
                            sc[:, i * 128:N], sc[:, i * 128:N], tril[:])
                        nc.scalar.activation(
                            sc[:, 0:N], sc[:, 0:N], AF.Exp,
                            accum_out=dn[i][:, h:h + 1])

            # ---------------- V projection (during the exp lag) -------------
            wv = load_w(wpb, "wv")
            vt = [wpa.tile([128, S], MMDT, name=f"v{c}", tag="w")
                  for c in range(C)]
            for c in range(C):
                proj_chunk(wv, c, bvt, vt[c])

            # ---------------- diag + denominators -> a ----------------
            nc.scalar.activation(diag_exp[:], dgacc[:], AF.Exp)
            nc.vector.tensor_mul(diag_exp[:], diag_exp[:], maskh[:])
            for i in range(T):
                tp = pp.tile([128, 512], F32, tag="mm")
                nc.tensor.transpose(tp[0:H, 0:128], dn[i][:], iden[:])
                nc.vector.tensor_copy(denomT[:, i * 128:(i + 1) * 128], tp[0:H, 0:128])
            nc.vector.reciprocal(arec[:], denomT[:])
            nc.vector.tensor_mul(a_t[:], diag_exp[:], arec[:])

            # ---------------- diagonal weighting of V ----------------
            for c in range(C):
                for (off, wd) in blocks(S):
                    abp = pp.tile([128, 512], F32, tag="mm")
                    nc.tensor.matmul(
                        abp[:, 0:wd],
                        ebc[c][:],
                        a_t[:, off:off + wd],
                        start=True,
                        stop=True,
                    )
                    nc.vector.tensor_mul(vt[c][:, off:off + wd],
                                         vt[c][:, off:off + wd], abp[:, 0:wd])

            # ---------------- output projection ----------------
            w0 = load_w(wpb, "w0")
            for m in range(T):
                for (off, wd) in blocks(D):
                    ps = pp.tile([128, 512], F32, tag="mm")
                    for c in range(C):
                        nc.tensor.matmul(
                            ps[:, 0:wd],
                            vt[c][:, m * 128:(m + 1) * 128],
                            w0[c][:, off:off + wd],
                            start=(c == 0),
                            stop=(c == C - 1),
                        )
                    ot = outp.tile([128, 512], F32, tag="o")
                    nc.vector.tensor_add(ot[:, 0:wd], ps[:, 0:wd], b0b[:, off:off + wd])
                    nc.sync.dma_start(
                        out_d[m * 128:(m + 1) * 128, off:off + wd], ot[:, 0:wd])

    nc.compile()
    return nc


def _get_nc():
    if "nc" not in _CACHE:
        _CACHE["nc"] = _build(S, D, H)
    return _CACHE["nc"]


def _host_aux(length):
    dk = D // H
    C = D // 128
    aux = {}
    aux["iden"] = np.eye(128, dtype=np.float32)
    tril = np.zeros((128, 128), np.float32)
    tril[np.triu_indices(128, 1)] = NEG
    aux["trilneg"] = tril
    ed = np.zeros((C, 128, H), np.float32)
    ebc = np.zeros((C, H, 128), np.float32)
    for c in range(C):
        for dl in range(128):
            h = (c * 128 + dl) // dk
            ed[c, dl, h] = 1.0
            ebc[c, h, dl] = 1.0
    aux["ed"] = ed
    aux["ebc"] = ebc
    mask = (np.arange(S) < int(length)).astype(np.float32)
    aux["maskh"] = np.tile(mask[None, :], (H, 1))
    return aux


def _in_map(x, wq, bq, wk, bk, wv, bv, w0, b0, length):
    C = D // 128
    inp = {"xt": np.ascontiguousarray(np.asarray(x, np.float32).T)}
    inp["wq"] = np.ascontiguousarray(wq, np.float32)
    inp["wk"] = np.ascontiguousarray(wk, np.float32)
    inp["wv"] = np.ascontiguousarray(wv, np.float32)
    inp["w0"] = np.ascontiguousarray(w0, np.float32)
    inp["bqt"] = np.ascontiguousarray(np.asarray(bq, np.float32).reshape(C, 128).T)
    inp["bkt"] = np.ascontiguousarray(np.asarray(bk, np.float32).reshape(C, 128).T)
    inp["bvt"] = np.ascontiguousarray(np.asarray(bv, np.float32).reshape(C, 128).T)
    inp["b0b"] = np.ascontiguousarray(
        np.tile(np.asarray(b0, np.float32)[None, :], (128, 1)))
    inp.update(_host_aux(length))
    return inp


def _run(inputs, trace=False):
    from concourse.bass_utils import run_bass_kernel_spmd

    batch = np.asarray(inputs["batch"], np.float32)
    lengths = np.asarray(inputs["lengths"])
    nb = batch.shape[0]
    assert batch.shape[1:] == (S, D), batch.shape
    nc = _get_nc()
    in_maps = [
        _in_map(batch[b], inputs["wq"], inputs["bq"], inputs["wk"], inputs["bk"],
                inputs["wv"], inputs["bv"], inputs["w0"], inputs["b0"],
                int(lengths[b]))
        for b in range(nb)
    ]
    res = run_bass_kernel_spmd(nc, in_maps, core_ids=list(range(nb)), trace=trace)
    out = np.stack([r["out"] for r in res.results]).astype(np.float32)
    return out, res


def kernel(**inputs) -> np.ndarray:
    out, _ = _run(inputs, trace=False)
    return out


# revision 31
# speedup vs baseline: 1.0549x; 1.0045x over previous
"""Trainium2 Bass kernel for nn_MultiHeadAttention_85761906966848 (sparse_attention).

Diagonal-only attention: out[b,s,:] = (softmax(masked scores)[s,s] * v) @ W0 + b0.
Data-parallel over batch (core b computes batch b end-to-end), as v1.

v3: same math/idioms as v1, restructured so the ScalarE exp stream (the
~130us critical resource) starts at ~15us instead of ~85us: Q and K project
PER DIM-CHUNK with that chunk's scores + tril + exp+accum emitted immediately
after, so exp overlaps the remaining projections.  Wq and Wk are both resident
(Wv/W0 rotate in later); V-proj runs during the scalar lag and writes into
Wq's dead buffers; the diag accumulates in SBUF to keep matmul accumulation
groups contiguous.
"""

import numpy as np
import concourse.bass as bass
import concourse.bacc as bacc
import concourse.mybir as mybir
from concourse import tile

F32 = mybir.dt.float32
F32R = mybir.dt.float32r
AF = mybir.ActivationFunctionType

B, S, D, H = 8, 1024, 1024, 16
NEG = -1.0e30

_CACHE = {}


def blocks(total, width=512):
    out = []
    off = 0
    while off < total:
        w = min(width, total - off)
        out.append((off, w))
        off += w
    return out


def _build(S=1024, D=1024, H=16):
    dk = D // H
    C = D // 128          # number of 128-row d-chunks
    T = S // 128          # number of 128-row s-tiles
    HPC = 128 // dk       # heads per chunk
    assert dk * H == D and C * 128 == D and T * 128 == S and HPC * dk == 128

    MMDT = F32R

    nc = bacc.Bacc("TRN2", target_bir_lowering=False, debug=False, num_devices=8)

    xt_d = nc.dram_tensor("xt", [D, S], MMDT, kind="ExternalInput")
    w_d = {}
    for wn in ("wq", "wk", "wv", "w0"):
        w_d[wn] = nc.dram_tensor(wn, [D, D], MMDT, kind="ExternalInput")
    bqt_d = nc.dram_tensor("bqt", [128, C], F32, kind="ExternalInput")
    bkt_d = nc.dram_tensor("bkt", [128, C], F32, kind="ExternalInput")
    bvt_d = nc.dram_tensor("bvt", [128, C], F32, kind="ExternalInput")
    b0b_d = nc.dram_tensor("b0b", [128, D], F32, kind="ExternalInput")
    ed_d = nc.dram_tensor("ed", [C, 128, H], MMDT, kind="ExternalInput")
    ebc_d = nc.dram_tensor("ebc", [C, H, 128], MMDT, kind="ExternalInput")
    mask_d = nc.dram_tensor("maskh", [H, S], F32, kind="ExternalInput")
    tril_d = nc.dram_tensor("trilneg", [128, 128], F32, kind="ExternalInput")
    iden_d = nc.dram_tensor("iden", [128, 128], F32, kind="ExternalInput")
    out_d = nc.dram_tensor("out", [S, D], F32, kind="ExternalOutput")

    with tile.TileContext(nc) as tc:
        with (
            tc.tile_pool(name="cp", bufs=1) as cp,
            tc.tile_pool(name="xtp", bufs=1) as xtp,
            tc.tile_pool(name="wpa", bufs=C) as wpa,
            tc.tile_pool(name="wpb", bufs=C) as wpb,
            tc.tile_pool(name="qkp", bufs=1) as qkp,
            tc.tile_pool(name="prodp", bufs=1) as prodp,
            tc.tile_pool(name="qzp", bufs=1) as qzp,
            tc.tile_pool(name="outp", bufs=2) as outp,
            tc.tile_pool(name="pp", bufs=2, space=bass.MemorySpace.PSUM) as pp,
            tc.tile_pool(name="pbig", bufs=3, space=bass.MemorySpace.PSUM) as pbig,
        ):
            # ---------------- constants ----------------
            iden = cp.tile([128, 128], F32, tag="iden")
            nc.sync.dma_start(iden[:], iden_d[:])
            tril = cp.tile([128, 128], F32, tag="tril")
            nc.sync.dma_start(tril[:], tril_d[:])
            b0b = cp.tile([128, D], F32, tag="b0b")
            nc.sync.dma_start(b0b[:], b0b_d[:])
            bqt = cp.tile([128, C], F32, tag="bqt")
            nc.sync.dma_start(bqt[:], bqt_d[:])
            bkt = cp.tile([128, C], F32, tag="bkt")
            nc.sync.dma_start(bkt[:], bkt_d[:])
            bvt = cp.tile([128, C], F32, tag="bvt")
            nc.sync.dma_start(bvt[:], bvt_d[:])
            maskh = cp.tile([H, S], F32, tag="maskh")
            nc.sync.dma_start(maskh[:], mask_d[:])
            ed = []
            ebc = []
            for c in range(C):
                e1 = cp.tile([128, H], MMDT, name=f"ed{c}", tag=f"ed{c}")
                nc.sync.dma_start(e1[:], ed_d[c, :, :])
                ed.append(e1)
                e2 = cp.tile([H, 128], MMDT, name=f"ebc{c}", tag=f"ebc{c}")
                nc.sync.dma_start(e2[:], ebc_d[c, :, :])
                ebc.append(e2)

            # persistent small result tiles
            diag_exp = cp.tile([H, S], F32, tag="diag_exp")
            denomT = cp.tile([H, S], F32, tag="denomT")
            arec = cp.tile([H, S], F32, tag="arec")
            a_t = cp.tile([H, S], MMDT, tag="a_t")
            dgacc = denomT  # diag accumulator is dead before denomT is written
            dn = [cp.tile([128, H], F32, name=f"dn{i}", tag=f"dn{i}") for i in range(T)]

            # ---------------- X^T load (host pre-transposed) ----------------
            xt = [xtp.tile([128, S], MMDT, name=f"xt{c}", tag=f"xt{c}") for c in range(C)]
            for (off, wd) in blocks(S):
                for c in range(C):
                    nc.sync.dma_start(xt[c][:, off:off + wd],
                                      xt_d[c * 128:(c + 1) * 128, off:off + wd])

            def load_w(pool, name):
                wts = [pool.tile([128, D], MMDT, name=f"{name}{c}", tag="w")
                       for c in range(C)]
                for (off, wd) in blocks(D):
                    for c in range(C):
                        nc.scalar.dma_start(
                            wts[c][:, off:off + wd],
                            w_d[name][c * 128:(c + 1) * 128, off:off + wd])
                return wts

            wq = load_w(wpa, "wq")
            wk = load_w(wpb, "wk")

            def proj_chunk(wts, dd, bias_tile, dst):
                for (off, wd) in blocks(S):
                    ps = pp.tile([128, 512], F32, tag="mm")
                    for kk in range(C):
                        nc.tensor.matmul(
                            ps[:, 0:wd],
                            wts[kk][:, dd * 128:(dd + 1) * 128],
                            xt[kk][:, off:off + wd],
                            start=(kk == 0),
                            stop=(kk == C - 1),
                        )
                    nc.vector.tensor_scalar_add(
                        dst[:, off:off + wd], ps[:, 0:wd],
                        bias_tile[:, dd:dd + 1])

            qt = [qkp.tile([128, S], MMDT, name=f"q{c}", tag=f"q{c}")
                  for c in range(C)]
            kt = [qkp.tile([128, S], MMDT, name=f"k{c}", tag=f"k{c}")
                  for c in range(C)]

            # persistent zero-padded Q staging: the dead half of each buffer
            # is zeroed once and never overwritten afterwards
            qzper = []
            for p in range(HPC):
                qz = qzp.tile([128, S], MMDT, name=f"qzp{p}", tag=f"qz{p}")
                zo = (1 - p) * dk
                nc.vector.tensor_scalar_mul(qz[zo:zo + dk, :],
                                            xt[0][zo:zo + dk, :], 0.0)
                qzper.append(qz)

            # ------- fused per-chunk: Q, K proj -> diag partials -> scores ---
            for c in range(C):
                proj_chunk(wq, c, bqt, qt[c])
                proj_chunk(wk, c, bkt, kt[c])

                # diag partial: dgacc += ed[c]^T @ (qt[c] * kt[c])
                for (off, wd) in blocks(S):
                    pr = prodp.tile([128, 512], MMDT, tag="prod")
                    nc.vector.tensor_mul(
                        pr[:, 0:wd], qt[c][:, off:off + wd], kt[c][:, off:off + wd])
                    dgc = pp.tile([128, 512], F32, tag="mm")
                    nc.tensor.matmul(
                        dgc[0:H, 0:wd],
                        ed[c][:],
                        pr[:, 0:wd],
                        start=True,
                        stop=True,
                    )
                    if c == 0:
                        nc.vector.tensor_copy(dgacc[:, off:off + wd],
                                              dgc[0:H, 0:wd])
                    else:
                        nc.vector.tensor_add(dgacc[:, off:off + wd],
                                             dgacc[:, off:off + wd],
                                             dgc[0:H, 0:wd])

                # scores for this chunk's two heads, zero-padded to K=128
                qzs = qzper
                for p in range(HPC):
                    nc.vector.tensor_copy(
                        qzs[p][p * dk:(p + 1) * dk, :],
                        qt[c][p * dk:(p + 1) * dk, :])
                for i in range(T):
                    N = (i + 1) * 128
                    for p in range(HPC):
                        h = c * HPC + p
                        sc = pbig.tile([128, 1024], F32, name=f"sc{p}", tag="big")
                        for (off, wd) in blocks(N):
                            nc.tensor.matmul(
                                sc[:, off:off + wd],
                                qzs[p][:, i * 128:(i + 1) * 128],
                                kt[c][:, off:off + wd],
                                start=True,
                                stop=True,
                            )
                        nc.vector.tensor_add(
                            sc[:, i * 128:N], sc[:, i * 128:N], tril[:])
                        nc.scalar.activation(
                            sc[:, 0:N], sc[:, 0:N], AF.Exp,
                            accum_out=dn[i][:, h:h + 1])

            # ---------------- V projection (during the exp lag) -------------
            wv = load_w(wpb, "wv")
            vt = [wpa.tile([128, S], MMDT, name=f"v{c}", tag="w")
                  for c in range(C)]
            for c in range(C):
                proj_chunk(wv, c, bvt, vt[c])

            w0 = load_w(wpb, "w0")

            # ---------------- diag + denominators -> a ----------------
            nc.scalar.activation(diag_exp[:], dgacc[:], AF.Exp)
            nc.vector.tensor_mul(diag_exp[:], diag_exp[:], maskh[:])
            for i in range(T):
                tp = pp.tile([128, 512], F32, tag="mm")
                nc.tensor.transpose(tp[0:H, 0:128], dn[i][:], iden[:])
                nc.vector.tensor_copy(denomT[:, i * 128:(i + 1) * 128], tp[0:H, 0:128])
            nc.vector.reciprocal(arec[:], denomT[:])
            nc.vector.tensor_mul(a_t[:], diag_exp[:], arec[:])

            # ---------------- diagonal weighting of V ----------------
            for c in range(C):
                for (off, wd) in blocks(S):
                    abp = pp.tile([128, 512], F32, tag="mm")
                    nc.tensor.matmul(
                        abp[:, 0:wd],
                        ebc[c][:],
                        a_t[:, off:off + wd],
                        start=True,
                        stop=True,
                    )
                    nc.vector.tensor_mul(vt[c][:, off:off + wd],
                                         vt[c][:, off:off + wd], abp[:, 0:wd])

            # ---------------- output projection ----------------
            for m in range(T):
                for (off, wd) in blocks(D):
                    ps = pp.tile([128, 512], F32, tag="mm")
                    for c in range(C):
                        nc.tensor.matmul(
                            ps[:, 0:wd],
                            vt[c][:, m * 128:(m + 1) * 128],
                            w0[c][:, off:off + wd],
                            start=(c == 0),
                            stop=(c == C - 1),
                        )
                    ot = outp.tile([128, 512], F32, tag="o")
                    nc.vector.tensor_add(ot[:, 0:wd], ps[:, 0:wd], b0b[:, off:off + wd])
                    nc.sync.dma_start(
                        out_d[m * 128:(m + 1) * 128, off:off + wd], ot[:, 0:wd])

    nc.compile()
    return nc


def _get_nc():
    if "nc" not in _CACHE:
        _CACHE["nc"] = _build(S, D, H)
    return _CACHE["nc"]


def _host_aux(length):
    dk = D // H
    C = D // 128
    aux = {}
    aux["iden"] = np.eye(128, dtype=np.float32)
    tril = np.zeros((128, 128), np.float32)
    tril[np.triu_indices(128, 1)] = NEG
    aux["trilneg"] = tril
    ed = np.zeros((C, 128, H), np.float32)
    ebc = np.zeros((C, H, 128), np.float32)
    for c in range(C):
        for dl in range(128):
            h = (c * 128 + dl) // dk
            ed[c, dl, h] = 1.0
            ebc[c, h, dl] = 1.0
    aux["ed"] = ed
    aux["ebc"] = ebc
    mask = (np.arange(S) < int(length)).astype(np.float32)
    aux["maskh"] = np.tile(mask[None, :], (H, 1))
    return aux


def _in_map(x, wq, bq, wk, bk, wv, bv, w0, b0, length):
    C = D // 128
    inp = {"xt": np.ascontiguousarray(np.asarray(x, np.float32).T)}
    inp["wq"] = np.ascontiguousarray(wq, np.float32)
    inp["wk"] = np.ascontiguousarray(wk, np.float32)
    inp["wv"] = np.ascontiguousarray(wv, np.float32)
    inp["w0"] = np.ascontiguousarray(w0, np.float32)
    inp["bqt"] = np.ascontiguousarray(np.asarray(bq, np.float32).reshape(C, 128).T)
    inp["bkt"] = np.ascontiguousarray(np.asarray(bk, np.float32).reshape(C, 128).T)
    inp["bvt"] = np.ascontiguousarray(np.asarray(bv, np.float32).reshape(C, 128).T)
    inp["b0b"] = np.ascontiguousarray(
        np.tile(np.asarray(b0, np.float32)[None, :], (128, 1)))
    inp.update(_host_aux(length))
    return inp


def _run(inputs, trace=False):
    from concourse.bass_utils import run_bass_kernel_spmd

    batch = np.asarray(inputs["batch"], np.float32)
    lengths = np.asarray(inputs["lengths"])
    nb = batch.shape[0]
    assert batch.shape[1:] == (S, D), batch.shape
    nc = _get_nc()
    in_maps = [
        _in_map(batch[b], inputs["wq"], inputs["bq"], inputs["wk"], inputs["bk"],
                inputs["wv"], inputs["bv"], inputs["w0"], inputs["b0"],
                int(lengths[b]))
        for b in range(nb)
    ]
    res = run_bass_kernel_spmd(nc, in_maps, core_ids=list(range(nb)), trace=trace)
    out = np.stack([r["out"] for r in res.results]).astype(np.float32)
    return out, res


def kernel(**inputs) -> np.ndarray:
    out, _ = _run(inputs, trace=False)
    return out
